# revision 1
# baseline (speedup 1.0000x reference)
import numpy as np

STACK, UNITS, D, EPS = 12, 4, 128, 1e-3
NPART = 128
T, NB = 16, 2048                 # t-blocks per group, cols per t-block
GROUPS = 2
G_ROWS = T * NB                  # 32768
NH = NB // 1024                  # rc rounds per stage (1024-col z tiles)


def _bf16(a):
    import ml_dtypes
    return np.asarray(a, dtype=ml_dtypes.bfloat16)


def prep_consts(inputs):
    """Host-side weight packing for the u-major T=16 layout."""
    ws = [np.asarray(inputs[f"w{i}"], np.float32) for i in range(STACK)]
    gamma = np.asarray(inputs["gamma"], np.float32)
    beta = np.asarray(inputs["beta"], np.float32)
    mean = np.asarray(inputs["mean"], np.float32)
    var = np.asarray(inputs["var"], np.float32)
    wf = np.asarray(inputs["wf"], np.float32)
    bf = np.asarray(inputs["bf"], np.float32)

    s = gamma / np.sqrt(var + EPS)
    bsh = beta - mean * s
    wd = wf[:, 0] - wf[:, 1]
    bd = float(bf[0] - bf[1])

    c = {}
    Wx = np.zeros((D, 49), np.float32)
    for i in range(STACK):
        for u in range(UNITS):
            Wx[:, 12 * u + i] = ws[i][4 * i:, u] * s[i, u]
    Wx[:, 48] = wd[48:]
    c["wx"] = _bf16(Wx)

    # A chunks: stage i, chunk cc = source stages {2cc, 2cc+1}
    for i in range(1, STACK):
        for cc in range(i // 2):
            M = np.zeros((128, 64), np.float32)
            for jj in range(2):
                j = 2 * cc + jj
                for v in range(4):
                    for u in range(4):
                        val = ws[i][4 * (i - 1 - j) + v, u] * s[i, u]
                        M[64 * jj + 16 * v:64 * jj + 16 * v + 16, 16 * u:16 * u + 16] \
                            [np.arange(16), np.arange(16)] = val
            c[f"a_{i}_{cc}"] = _bf16(M)
        if i % 2 == 1:
            j = i - 1
            M = np.zeros((64, 64), np.float32)
            for v in range(4):
                for u in range(4):
                    val = ws[i][4 * (i - 1 - j) + v, u] * s[i, u]
                    M[16 * v:16 * v + 16, 16 * u:16 * u + 16][np.arange(16), np.arange(16)] = val
            c[f"ap_{i}"] = _bf16(M)

    for cc in range(6):
        M = np.zeros((128, 64), np.float32)
        for jj in range(2):
            j = 2 * cc + jj
            for v in range(4):
                val = wd[4 * (11 - j) + v]
                for u in range(4):
                    M[64 * jj + 16 * v:64 * jj + 16 * v + 16, 16 * u:16 * u + 16] \
                        [np.arange(16), np.arange(16)] = val
        c[f"wd_{cc}"] = _bf16(M)

    c["s_id"] = _bf16(np.eye(64, dtype=np.float32))
    S_d = np.zeros((64, 64), np.float32)
    for t in range(T):
        for u in range(4):
            S_d[t, 16 * u + t] = 1.0
    c["s_d"] = _bf16(S_d)

    B = np.zeros((64, STACK), np.float32)
    for i in range(STACK):
        for u in range(4):
            B[16 * u:16 * u + 16, i] = bsh[i, u]
    c["bias"] = B
    c["bd"] = bd

    # pack the many small stationaries into 3 big tensors (3 DMA loads
    # instead of ~45 -- the per-DMA issue cost was delaying the first
    # scatters and with them the whole first recurrence)
    a_keys = [f"a_{i}_{cc}" for i in range(1, STACK) for cc in range(i // 2)]
    c2 = {"biga": np.concatenate([c.pop(k) for k in a_keys]
                                 + [c.pop(f"wd_{cc}") for cc in range(6)], axis=1)}
    s_keys = [f"ap_{i}" for i in range(1, STACK, 2)] + ["s_id", "s_d"]
    c2["bigs"] = np.concatenate([c.pop(k) for k in s_keys], axis=1)
    c2["wx"] = c.pop("wx")
    c2["bias"] = c.pop("bias")
    c2["bd"] = c.pop("bd")
    return c2


def build_kernel(ctx, tc, outs, ins, *, bd):
    import concourse.mybir as mybir

    nc = tc.nc
    f32 = mybir.dt.float32
    bf16 = mybir.dt.bfloat16
    ACT = mybir.ActivationFunctionType
    ALU = mybir.AluOpType

    x_ap = ins["x"]
    out_ap = outs["out"]

    const_pool = ctx.enter_context(tc.tile_pool(name="consts", bufs=1))

    def load_const(name, shape, dt=f32):
        t = const_pool.tile(list(shape), dt, tag=name, name=name)
        nc.gpsimd.dma_start(t[:], ins[name])
        return t

    wx_sb = load_const("wx", (D, 49), bf16)
    n_a = sum(i // 2 for i in range(1, STACK))            # 30
    biga = load_const("biga", (128, (n_a + 6) * 64), bf16)
    bigs = load_const("bigs", (64, 8 * 64), bf16)
    a_sb = {}
    k = 0
    for i in range(1, STACK):
        for cc in range(i // 2):
            a_sb[(i, cc)] = biga[:, k * 64:(k + 1) * 64]
            k += 1
    wd_sb = [biga[:, (n_a + cc) * 64:(n_a + cc + 1) * 64] for cc in range(6)]
    ap_sb = {i: bigs[:, k * 64:(k + 1) * 64]
             for k, i in enumerate(range(1, STACK, 2))}
    sid_sb = bigs[:, 6 * 64:7 * 64]
    sd_sb = bigs[:, 7 * 64:8 * 64]
    bias_sb = load_const("bias", (64, STACK))

    xt_pool = ctx.enter_context(tc.tile_pool(name="xt", bufs=4))
    cx_pool = ctx.enter_context(tc.tile_pool(name="cx", bufs=4))
    cxf_pool = ctx.enter_context(tc.tile_pool(name="cxf", bufs=2))
    cxfd_pool = ctx.enter_context(tc.tile_pool(name="cxfd", bufs=2))
    y2_pool = ctx.enter_context(tc.tile_pool(name="y2", bufs=2))
    out_pool = ctx.enter_context(tc.tile_pool(name="outsb", bufs=2))

    pcx_pool = ctx.enter_context(tc.tile_pool(name="pcx", bufs=2, space="PSUM"))
    z_pool = ctx.enter_context(tc.tile_pool(name="z", bufs=4, space="PSUM"))

    # Per-group state (tiles), created lazily by the pipeline below.
    state = {}

    def start_group(g):
        cxF = cxf_pool.tile([64, STACK * NB], bf16, tag="cxF", name="cxF")
        cxFd = cxfd_pool.tile([64, NB], bf16, tag="cxFd", name="cxFd")
        # rows 16..64 of cxFd feed the K=64 d-inject; zero everything first
        # (scatters then overwrite rows 0..15)
        nc.vector.memset(cxFd[:], 0.0)
        y2 = [y2_pool.tile([128, NB], bf16, tag=f"y2c{cc}", name=f"y2c{cc}")
              for cc in range(6)]
        state[g] = (cxF, cxFd, y2)

    def emit_xtile(g, t):
        cxF, cxFd, y2 = state[g]
        r0 = g * G_ROWS + t * NB
        xt = xt_pool.tile([NPART, NB], bf16, tag="xt")
        eng = nc.sync if t % 2 == 0 else nc.scalar
        eng.dma_start(xt[:], x_ap[r0:r0 + NB, :], transpose=True)
        cx = cx_pool.tile([49, NB], bf16, tag="cx")
        for rc in range(NH):
            pcx = pcx_pool.tile([49, 1024], f32, tag="pcx")
            for h in range(2):
                nc.tensor.matmul(
                    pcx[:, h * 512:(h + 1) * 512], wx_sb[:],
                    xt[:, rc * 1024 + h * 512: rc * 1024 + (h + 1) * 512],
                    start=True, stop=True,
                )
            dst = cx[:, rc * 1024:(rc + 1) * 1024]
            if (t + rc) % 2 == 0:
                nc.scalar.activation(dst, pcx[:], ACT.Copy)
            else:
                nc.vector.tensor_copy(dst, pcx[:])
        # one-DMA scatter: rows 12u+i -> cxF[16u+t, block i]
        ed = cxF[:].rearrange("(u s) (i n) -> u s i n", u=4, i=STACK)[:, t]
        nc.gpsimd.dma_start(ed, cx[0:48, :])
        nc.scalar.dma_start(cxFd[t:t + 1, :], cx[48:49, :])

    def emit_stage(g, i):
        cxF, cxFd, y2 = state[g]
        # per-rc z tiles (1 PSUM bank each): relu of the rc0 half can retire
        # while rc1 matmuls still run, unblocking stage i+1's rc0 chunks early
        zs = [z_pool.tile([128, 512], f32, tag="z", name=f"z{rc}")
              for rc in range(NH)]

        def slices(rc, cb):
            zsl = zs[rc][cb * 64:(cb + 1) * 64, :]
            ysl = slice(rc * 1024 + cb * 512, rc * 1024 + (cb + 1) * 512)
            return zsl, ysl

        ncc = i // 2
        has_part = (i % 2 == 1)
        # stationary-major emission: 4 matmuls (rc x cb) per weight load
        for rc in range(NH):
            for cb in range(2):
                zsl, ysl = slices(rc, cb)
                csl = slice(i * NB + rc * 1024 + cb * 512,
                            i * NB + rc * 1024 + (cb + 1) * 512)
                nc.tensor.matmul(zsl, sid_sb[:], cxF[0:64, csl],
                                 start=True, stop=(ncc == 0 and not has_part))
        if has_part:
            for rc in range(NH):
                for cb in range(2):
                    zsl, ysl = slices(rc, cb)
                    nc.tensor.matmul(zsl, ap_sb[i][:], y2[i // 2][0:64, ysl],
                                     start=False, stop=(ncc == 0))
        for cc in range(ncc):
            for rc in range(NH):
                for cb in range(2):
                    zsl, ysl = slices(rc, cb)
                    nc.tensor.matmul(zsl, a_sb[(i, cc)][:], y2[cc][:, ysl],
                                     start=False, stop=(cc == ncc - 1))
        # relu + bias -> y2 slice, per (rc, cb)
        ch, half = i // 2, 64 * (i % 2)
        for rc in range(NH):
            for cb in range(2):
                src = zs[rc][cb * 64:(cb + 1) * 64, :]
                dst = y2[ch][half:half + 64,
                             rc * 1024 + cb * 512: rc * 1024 + (cb + 1) * 512]
                if (i + rc + cb) % 2 == 0:
                    nc.scalar.activation(dst, src, ACT.Relu, bias=bias_sb[:, i:i + 1])
                else:
                    nc.vector.tensor_scalar(dst, src, bias_sb[:, i:i + 1], 0.0,
                                            ALU.add, ALU.max)

    def emit_tail(g):
        cxF, cxFd, y2 = state[g]
        pds = [z_pool.tile([128, 512], f32, tag="z", name=f"pd{rc}")
               for rc in range(NH)]
        for rc in range(NH):
            for cb in range(2):
                psl = pds[rc][cb * 64:(cb + 1) * 64, :]
                dsl = slice(rc * 1024 + cb * 512, rc * 1024 + (cb + 1) * 512)
                nc.tensor.matmul(psl, sd_sb[:], cxFd[0:64, dsl],
                                 start=True, stop=False)
        for cc in range(6):
            for rc in range(NH):
                for cb in range(2):
                    psl = pds[rc][cb * 64:(cb + 1) * 64, :]
                    ysl = slice(rc * 1024 + cb * 512, rc * 1024 + (cb + 1) * 512)
                    nc.tensor.matmul(psl, wd_sb[cc][:], y2[cc][:, ysl],
                                     start=False, stop=(cc == 5))
        outsb = out_pool.tile([128, NB], f32, tag="outsb")
        o4 = outsb[:].rearrange("p (rc n two) -> p rc n two", rc=NH, two=2)
        for rc in range(NH):
            nc.scalar.activation(o4[:, rc, :, 0], pds[rc][:], ACT.Sigmoid,
                                 bias=float(bd))
            nc.scalar.activation(o4[:, rc, :, 1], pds[rc][:], ACT.Sigmoid,
                                 bias=float(-bd), scale=-1.0)
        og = out_ap[g * G_ROWS:(g + 1) * G_ROWS, :].rearrange(
            "(t rc c n) two -> c t rc (n two)", rc=NH, c=2, n=512)
        for cb in range(2):
            osrc = outsb[cb * 64:cb * 64 + T, :].rearrange("p (rc f) -> p rc f", rc=NH)
            nc.gpsimd.dma_start(og[cb], osrc)

    # Software pipeline: group g's recurrence interleaves group g+1's x-tiles
    # so the PE never drains (keeps the HAM clock warm). Group g's tail (wd
    # chain) is deferred into group g+1's early stages for the same reason.
    start_group(0)
    for t in range(T):
        emit_xtile(0, t)
    pending_tail = None
    for g in range(GROUPS):
        if g + 1 < GROUPS:
            start_group(g + 1)
        emitted = 0
        for i in range(STACK):
            emit_stage(g, i)
            if pending_tail is not None:
                emit_tail(pending_tail)
                pending_tail = None
            if g + 1 < GROUPS:
                want = (i + 1) * T // STACK
                while emitted < want:
                    emit_xtile(g + 1, emitted)
                    emitted += 1
        pending_tail = g
    emit_tail(pending_tail)


# ---------------------------------------------------------------------------
# Self-contained entry point: kernel(**inputs) -> [500000, 2] float32
# ---------------------------------------------------------------------------

import sys as _sys
if '/opt/trn_rl_repo' not in _sys.path:
    _sys.path.insert(0, '/opt/trn_rl_repo')

B_FULL = 500000
N_CORES = 8
CORE_ROWS = GROUPS * G_ROWS                      # 65536
B_PAD = CORE_ROWS * N_CORES                      # 524288

_CACHE = {}


def _build_nc(const_shapes, bd):
    from contextlib import ExitStack
    import concourse.mybir as mybir
    from concourse import bacc
    import concourse.tile as tile

    nc = bacc.Bacc("TRN2", target_bir_lowering=False, debug=False,
                   num_devices=N_CORES)
    ins = {}
    ins["x"] = nc.dram_tensor("x", [CORE_ROWS, D], mybir.dt.bfloat16,
                              kind="ExternalInput").ap()
    for name, shp, npdt in const_shapes:
        dt = mybir.dt.bfloat16 if npdt == 'bfloat16' else mybir.dt.float32
        ins[name] = nc.dram_tensor(name, list(shp), dt,
                                   kind="ExternalInput").ap()
    outs = {"out": nc.dram_tensor("out", [CORE_ROWS, 2], mybir.dt.float32,
                                  kind="ExternalOutput").ap()}
    with tile.TileContext(nc) as tc:
        with ExitStack() as ctx:
            build_kernel(ctx, tc, outs, ins, bd=bd)
    nc.compile()
    return nc


def kernel(**inputs):
    import numpy as np
    import ml_dtypes
    from concourse.bass_utils import run_bass_kernel_spmd

    consts = prep_consts(inputs)
    bd = consts.pop("bd")
    x = np.asarray(inputs["x"], dtype=np.float32)
    assert x.shape == (B_FULL, D)
    xp = np.zeros((B_PAD, D), ml_dtypes.bfloat16)
    xp[:B_FULL] = x.astype(ml_dtypes.bfloat16)

    key = "nc"
    if key not in _CACHE:
        shapes = tuple((k, v.shape, str(v.dtype)) for k, v in consts.items())
        _CACHE[key] = _build_nc(shapes, bd)
    nc = _CACHE[key]

    in_maps = []
    for c in range(N_CORES):
        m = {"x": xp[c * CORE_ROWS:(c + 1) * CORE_ROWS]}
        m.update(consts)
        in_maps.append(m)
    res = run_bass_kernel_spmd(nc, in_maps, core_ids=list(range(N_CORES)))
    out = np.concatenate([res.results[c]["out"] for c in range(N_CORES)], axis=0)
    return out[:B_FULL]



# revision 14
# speedup vs baseline: 1.8466x; 1.8466x over previous
import numpy as np

STACK, UNITS, D, EPS = 12, 4, 128, 1e-3
T = 16                    # t-blocks per group (partition dim: 4 units x 16 t)
NB = 1024                 # cols per t-block
G_ROWS = T * NB           # 16384 rows per group
GROUPS = 4
NPAIR = STACK // 2        # 6 stage pairs
CORE_ROWS = GROUPS * G_ROWS   # 65536
B_FULL = 500000
N_CORES = 8
B_PAD = CORE_ROWS * N_CORES   # 524288


def _bf16(a):
    import ml_dtypes
    return np.asarray(a, np.float32).astype(ml_dtypes.bfloat16)


def _const_layout():
    """Column offsets of each stationary inside the packed `big` tensor."""
    off = {}
    c = 0
    def add(name, w):
        nonlocal c
        off[name] = c
        c += w
    add("wx", 49)
    add("ident", 128)
    for q in range(1, NPAIR):
        for cc in range(q):
            add(f"a_{q}_{cc}", 128)
    for q in range(NPAIR):
        add(f"p_{q}", 64)
    for cc in range(NPAIR):
        add(f"wd_{cc}", 16)
    add("sd", 16)
    return off, c


def prep_consts(inputs):
    """Host-side packing of all stationaries for the pair-fused layout.

    Data layouts on device:
      cx tile  [128, NB]: row 64k + 12u + i  (k: t-block within pcx tile,
               i = 2q+h), row 64k+48 = x-part of the final logit diff.
      cxF      [128, NPAIR*NB]: partition 64h + 16u + t, block q of NB cols;
               holds scale*(x @ w_i) + bias for stage i = 2q + h.
      y2[cc]   [128, NB]: partition 64h + 16v + t = relu output of stage 2cc+h.
    """
    ws = [np.asarray(inputs[f"w{i}"], np.float32) for i in range(STACK)]
    gamma = np.asarray(inputs["gamma"], np.float32)
    beta = np.asarray(inputs["beta"], np.float32)
    mean = np.asarray(inputs["mean"], np.float32)
    var = np.asarray(inputs["var"], np.float32)
    wf = np.asarray(inputs["wf"], np.float32)
    bf = np.asarray(inputs["bf"], np.float32)

    s = gamma / np.sqrt(var + EPS)
    bsh = beta - mean * s
    wd = wf[:, 0] - wf[:, 1]
    bd = float(bf[0] - bf[1])

    off, width = _const_layout()
    big = np.zeros((128, width), np.float32)
    rng16 = np.arange(16)

    # x-projection weights: col 24h + 6u + q for stage i = 2q + h
    # (order chosen so the cx->cxF scatter balances to 3 AP dims);
    # col 48 = x part of logit diff
    o = off["wx"]
    for i in range(STACK):
        q, h = divmod(i, 2)
        for u in range(UNITS):
            big[:, o + 24 * h + 6 * u + q] = ws[i][4 * i:, u] * s[i, u]
    big[:, o + 48] = wd[4 * STACK:]

    big[:, off["ident"]:off["ident"] + 128] = np.eye(128)

    # chunk stationaries: src pair cc (stages 2cc+jj) -> dest pair q (stages 2q+h)
    for q in range(1, NPAIR):
        for cc in range(q):
            o = off[f"a_{q}_{cc}"]
            for jj in range(2):
                j = 2 * cc + jj
                for h in range(2):
                    i = 2 * q + h
                    for v in range(UNITS):
                        for u in range(UNITS):
                            val = ws[i][4 * (i - 1 - j) + v, u] * s[i, u]
                            big[64 * jj + 16 * v + rng16,
                                o + 64 * h + 16 * u + rng16] = val

    # intra-pair partials: y_{2q} -> stage 2q+1 (dest cols 16u+t, M=64)
    for q in range(NPAIR):
        i = 2 * q + 1
        o = off[f"p_{q}"]
        for v in range(UNITS):
            for u in range(UNITS):
                val = ws[i][v, u] * s[i, u]
                big[16 * v + rng16, o + 16 * u + rng16] = val

    # tail: wd coefficients, src pair cc -> single logit-diff row (cols = t)
    for cc in range(NPAIR):
        o = off[f"wd_{cc}"]
        for jj in range(2):
            j = 2 * cc + jj
            for v in range(UNITS):
                big[64 * jj + 16 * v + rng16, o + rng16] = wd[4 * (STACK - 1 - j) + v]

    # d-inject identity (cxFd row t -> pds row t)
    big[rng16, off["sd"] + rng16] = 1.0

    # per-partition bias folded into the pcx->cx copy: row 64k + 24h + 6u + q
    bias = np.zeros((128, 1), np.float32)
    for k in range(2):
        for u in range(UNITS):
            for i in range(STACK):
                q, h = divmod(i, 2)
                bias[64 * k + 24 * h + 6 * u + q, 0] = bsh[i, u]

    return {"big": _bf16(big), "bias": bias, "bd": bd}


DEBUG = False


def build_kernel(ctx, tc, outs, ins, *, bd):
    import concourse.mybir as mybir

    nc = tc.nc
    f32 = mybir.dt.float32
    bf16 = mybir.dt.bfloat16
    ACT = mybir.ActivationFunctionType
    ALU = mybir.AluOpType

    xg_ap = ins["x"]          # [128, CORE_ROWS] bf16 (pre-transposed on host)
    out_ap = outs["out"]      # [CORE_ROWS, 2] f32

    off, width = _const_layout()
    const_pool = ctx.enter_context(tc.tile_pool(name="consts", bufs=1))
    big = const_pool.tile([128, width], bf16, tag="big", name="big")
    nc.sync.dma_start(big[:], ins["big"])
    bias_sb = const_pool.tile([128, 1], f32, tag="bias", name="bias_sb")
    nc.sync.dma_start(bias_sb[:], ins["bias"])

    wx = big[:, off["wx"]:off["wx"] + 49]
    ident = big[:, off["ident"]:off["ident"] + 128]
    a_st = {(q, cc): big[:, off[f"a_{q}_{cc}"]:off[f"a_{q}_{cc}"] + 128]
            for q in range(1, NPAIR) for cc in range(q)}
    p_st = {q: big[0:64, off[f"p_{q}"]:off[f"p_{q}"] + 64] for q in range(NPAIR)}
    wd_st = [big[:, off[f"wd_{cc}"]:off[f"wd_{cc}"] + 16] for cc in range(NPAIR)]
    sd = big[0:16, off["sd"]:off["sd"] + 16]

    xt_pool = ctx.enter_context(tc.tile_pool(name="xt", bufs=6))
    cx_pool = ctx.enter_context(tc.tile_pool(name="cx", bufs=6))
    cxf_pool = ctx.enter_context(tc.tile_pool(name="cxf", bufs=2))
    cxfd_pool = ctx.enter_context(tc.tile_pool(name="cxfd", bufs=2))
    y2_pool = ctx.enter_context(tc.tile_pool(name="y2", bufs=2))
    out_pool = ctx.enter_context(tc.tile_pool(name="outsb", bufs=2))
    psum_pool = ctx.enter_context(tc.tile_pool(name="ps", bufs=2, space="PSUM"))

    state = {}

    def start_group(g):
        cxF = cxf_pool.tile([128, NPAIR * NB], bf16, tag="cxF", name="cxF")
        cxFd = cxfd_pool.tile([16, NB], bf16, tag="cxFd", name="cxFd")
        y2 = [y2_pool.tile([128, NB], bf16, tag=f"y2c{cc}", name=f"y2c{cc}")
              for cc in range(NPAIR)]
        state[g] = (cxF, cxFd, y2)

    # engine rotation for element-wise work (scalar = ACT engine, vector = DVE)
    def relu(idx, dst, src):
        if idx % 2 == 0:
            nc.scalar.activation(dst, src, ACT.Relu)
        else:
            nc.vector.tensor_scalar(dst, src, 0.0, None, ALU.max)

    def emit_cx_tile(g, j):
        """x-projection for t-blocks 2j, 2j+1 of group g."""
        cxF, cxFd, y2 = state[g]
        xt = xt_pool.tile([128, 2 * NB], bf16, tag="xt")
        c0 = g * G_ROWS + j * 2 * NB
        nc.sync.dma_start(xt[:], xg_ap[:, c0:c0 + 2 * NB])
        pcx = psum_pool.tile([128, NB], f32, tag="ps", name="pcx")
        for tb in range(2):
            for hh in range(2):
                nc.tensor.matmul(
                    pcx[64 * tb:64 * tb + 49, 512 * hh:512 * hh + 512],
                    wx, xt[:, NB * tb + 512 * hh:NB * tb + 512 * hh + 512],
                    start=True, stop=True)
        # NB+64 pad: keeps the scatter's q-stride (one partition) from being
        # dim-merged with the 1024-element inner run into a bogus contiguous
        # 6144-element descriptor that would read past the partition.
        cxsb_full = cx_pool.tile([128, NB + 64], bf16, tag="cx")
        cxsb = cxsb_full[:, 0:NB]
        if j % 2 == 0:
            nc.scalar.activation(cxsb[:], pcx[:], ACT.Identity, bias=bias_sb[:, 0:1])
        else:
            nc.vector.tensor_scalar(cxsb[:], pcx[:], bias_sb[:, 0:1], None, ALU.add)
        # scatter rows 64k + 24h + 6u + q -> cxF[16*(4h+u) + t], block q.
        # src must stay 2D (the balancer splits 48 -> 8x6); an explicit 3D
        # src AP silently drops the middle dim in SW-DGE descriptor gen.
        for k in range(2):
            dst = cxF[:].rearrange(
                "(p s) (q n) -> p q s n", p=8, s=T, q=NPAIR)[:, :, 2 * j + k]
            nc.gpsimd.dma_start(dst, cxsb[64 * k:64 * k + 48])
        dsrc = cxsb[:].rearrange("(k c) n -> k c n", k=2)[:, 48]
        nc.sync.dma_start(cxFd[2 * j:2 * j + 2, :], dsrc)

    def emit_pair(g, q):
        cxF, cxFd, y2 = state[g]
        z = psum_pool.tile([128, NB], f32, tag="z", name=f"z{q}")
        csl = slice(q * NB, (q + 1) * NB)
        if q == 0:
            # stage 0 is pure relu(cx): read cxF directly, PSUM only for stage 1
            for hh in range(2):
                nc.tensor.matmul(z[64:128, 512 * hh:512 * hh + 512],
                                 ident[:, 64:128],
                                 cxF[:, q * NB + 512 * hh:q * NB + 512 * hh + 512],
                                 start=True, stop=False)
            relu(0, y2[0][0:64, :], cxF[0:64, csl])
        else:
            for hh in range(2):
                nc.tensor.matmul(z[:, 512 * hh:512 * hh + 512], ident,
                                 cxF[:, q * NB + 512 * hh:q * NB + 512 * hh + 512],
                                 start=True, stop=False)
            for cc in range(q):
                for hh in range(2):
                    nc.tensor.matmul(z[:, 512 * hh:512 * hh + 512], a_st[(q, cc)],
                                     y2[cc][:, 512 * hh:512 * hh + 512],
                                     start=False, stop=False,
                                     skip_group_check=True)
            relu(q, y2[q][0:64, :], z[0:64, :])
        for hh in range(2):
            nc.tensor.matmul(z[64:128, 512 * hh:512 * hh + 512], p_st[q],
                             y2[q][0:64, 512 * hh:512 * hh + 512],
                             start=False, stop=True, skip_group_check=True)
        relu(q + 1, y2[q][64:128, :], z[64:128, :])

    def emit_tail(g):
        cxF, cxFd, y2 = state[g]
        pds = psum_pool.tile([128, NB], f32, tag="ps", name="pds")
        for hh in range(2):
            nc.tensor.matmul(pds[0:16, 512 * hh:512 * hh + 512], sd,
                             cxFd[:, 512 * hh:512 * hh + 512],
                             start=True, stop=False)
        for cc in range(NPAIR):
            for hh in range(2):
                nc.tensor.matmul(pds[0:16, 512 * hh:512 * hh + 512], wd_st[cc],
                                 y2[cc][:, 512 * hh:512 * hh + 512],
                                 start=False, stop=(cc == NPAIR - 1),
                                 skip_group_check=True)
        outsb = out_pool.tile([16, 2 * NB], f32, tag="outsb")
        o2 = outsb[:].rearrange("p (n two) -> p n two", two=2)
        nc.scalar.activation(o2[:, :, 0], pds[0:16, :], ACT.Sigmoid, bias=float(bd))
        nc.scalar.activation(o2[:, :, 1], pds[0:16, :], ACT.Sigmoid,
                             bias=float(-bd), scale=-1.0)
        og = out_ap[g * G_ROWS:(g + 1) * G_ROWS, :].rearrange(
            "(t n) two -> t (n two)", t=T)
        nc.scalar.dma_start(og, outsb[:])

    def emit_debug(g):
        cxF, cxFd, y2 = state[g]
        nc.sync.dma_start(outs["dbg_cxf"], cxF[:])
        nc.sync.dma_start(outs["dbg_cxfd"], cxFd[:])
        for cc in range(NPAIR):
            nc.sync.dma_start(outs["dbg_y2"][:, cc * NB:(cc + 1) * NB], y2[cc][:])

    # Software pipeline: group g's pair recurrence interleaves group g+1's
    # x-projection tiles so the PE never drains; tails are deferred one group.
    start_group(0)
    for j in range(8):
        emit_cx_tile(0, j)
    pending_tail = None
    for g in range(GROUPS):
        if g + 1 < GROUPS:
            start_group(g + 1)
        emitted = 0
        for q in range(NPAIR):
            emit_pair(g, q)
            if pending_tail is not None:
                emit_tail(pending_tail)
                pending_tail = None
            if g + 1 < GROUPS:
                want = (q + 1) * 8 // NPAIR
                while emitted < min(want, 8):
                    emit_cx_tile(g + 1, emitted)
                    emitted += 1
        pending_tail = g
        if DEBUG and g == 0:
            emit_debug(0)
    emit_tail(pending_tail)


# ---------------------------------------------------------------------------
# Self-contained entry point: kernel(**inputs) -> [500000, 2] float32
# ---------------------------------------------------------------------------

import sys as _sys
if '/opt/trn_rl_repo' not in _sys.path:
    _sys.path.insert(0, '/opt/trn_rl_repo')

_CACHE = {}


def _build_nc(bd):
    from contextlib import ExitStack
    import concourse.mybir as mybir
    from concourse import bacc
    import concourse.tile as tile

    _, width = _const_layout()
    nc = bacc.Bacc("TRN2", target_bir_lowering=False, debug=False,
                   num_devices=N_CORES)
    ins = {}
    ins["x"] = nc.dram_tensor("x", [128, CORE_ROWS], mybir.dt.bfloat16,
                              kind="ExternalInput").ap()
    ins["big"] = nc.dram_tensor("big", [128, width], mybir.dt.bfloat16,
                                kind="ExternalInput").ap()
    ins["bias"] = nc.dram_tensor("bias", [128, 1], mybir.dt.float32,
                                 kind="ExternalInput").ap()
    outs = {"out": nc.dram_tensor("out", [CORE_ROWS, 2], mybir.dt.float32,
                                  kind="ExternalOutput").ap()}
    if DEBUG:
        outs["dbg_cxf"] = nc.dram_tensor(
            "dbg_cxf", [128, NPAIR * NB], mybir.dt.bfloat16,
            kind="ExternalOutput").ap()
        outs["dbg_cxfd"] = nc.dram_tensor(
            "dbg_cxfd", [16, NB], mybir.dt.bfloat16, kind="ExternalOutput").ap()
        outs["dbg_y2"] = nc.dram_tensor(
            "dbg_y2", [128, NPAIR * NB], mybir.dt.bfloat16,
            kind="ExternalOutput").ap()
    with tile.TileContext(nc) as tc:
        with ExitStack() as ctx:
            build_kernel(ctx, tc, outs, ins, bd=bd)
    nc.compile()
    return nc


def make_in_maps(inputs):
    import ml_dtypes

    consts = prep_consts(inputs)
    bd = consts.pop("bd")
    x = np.asarray(inputs["x"], dtype=np.float32)
    assert x.shape == (B_FULL, D)
    xb = np.zeros((B_PAD, D), ml_dtypes.bfloat16)
    xb[:B_FULL] = x.astype(ml_dtypes.bfloat16)

    in_maps = []
    for c in range(N_CORES):
        xt = np.ascontiguousarray(xb[c * CORE_ROWS:(c + 1) * CORE_ROWS].T)
        m = {"x": xt}
        m.update(consts)
        in_maps.append(m)
    return in_maps, bd


def kernel(**inputs):
    from concourse.bass_utils import run_bass_kernel_spmd

    in_maps, bd = make_in_maps(inputs)
    if "nc" not in _CACHE:
        _CACHE["nc"] = _build_nc(bd)
    nc = _CACHE["nc"]
    res = run_bass_kernel_spmd(nc, in_maps, core_ids=list(range(N_CORES)))
    out = np.concatenate([res.results[c]["out"] for c in range(N_CORES)], axis=0)
    return out[:B_FULL]
